# Initial kernel scaffold
#
"""Trainium2 Bass kernel for nn_CausalDit (sparse frame-causal DiT).

Sharding over 8 NeuronCores (SPMD, one program):
- Token space: 11 frames (6 noisy "zr" + 5 clean "xa"), 257 tokens each
  (256 patch tokens + 1 register/action token).
- Dense compute (LN/AdaLN-mod, QKV, Wo, GEGLU FFN, gates) is sharded by
  tokens: core c owns rows [32c, 32c+32) of every frame; the per-frame
  257th token ("leftover") is computed replicated on every core.
  Per-core stream: [11 frames x 33 rows, 512] = [363, 512].
- Attention is sharded by head (8 heads <-> 8 cores), block-sparse at
  frame granularity (no masks): an AllToAll redistributes Q^T/K^T/V from
  token-shards to head-shards, each core runs all 35 frame-pair attention
  units for its head (softmax without max-subtraction; denominator via a
  ones-column appended to V), and a second AllToAll returns per-head
  outputs + softmax denominators to token owners for the Wo projection.
- Weights are replicated in bf16; matmuls run bf16 on the PE with fp32
  PSUM accumulation; LN/softmax math in fp32.
- Host (numpy, fp32) does only the tiny prep: patchify + patch matmul,
  embedding gathers, per-frame AdaLN scale/shift/gate tables, bias folds
  (b_k dropped - softmax-shift-invariant; b_v folded into b_o), and the
  final unpatch matmul.
"""
import numpy as np
import ml_dtypes

import concourse.bass as bass
import concourse.mybir as mybir
import concourse.tile as tile
from concourse import bacc
from concourse.bass_utils import run_bass_kernel_spmd
from concourse.masks import make_identity

# ---- model constants (hardcoded from the problem spec) ----
P2 = 2; NH = 8; NW = 4; NB = 6; D = 512; HID = 2048; NREG = 1; NACT = 3
HH = 32; WW = 32; C = 3; B = 1; DUR = 6
DH = D // NH          # 64
NZ = DUR              # 6 zr frames
NX = DUR - 1          # 5 xa frames
NF = NZ + NX          # 11 frames
S = (HH // P2) * (WW // P2)   # 256
TPF = S + 1           # 257 tokens/frame
NCORE = 8
OWN = 32              # owned rows per frame per core
TOK = NF * (OWN + 1)  # 363 rows per core
CORE_IDS = list(range(NCORE))
TOKT = [(0, 128), (128, 128), (256, 107)]   # token tiles of 363
F32 = mybir.dt.float32
BF16 = mybir.dt.bfloat16
BF = ml_dtypes.bfloat16
AX = mybir.AxisListType.X
ALU = mybir.AluOpType
ACT = mybir.ActivationFunctionType


def _kv_frames(fq):
    """Global kv-frame indices for q-frame fq (zr: 0..5, xa: 6..10)."""
    if fq < NZ:
        return [fq] + [NZ + j for j in range(max(0, fq - NW), min(fq, NX))]
    return list(range(NZ, fq + 1))


def _runs(fq):
    """Valid leftover-kv rows as [a, b) runs in the global frame index."""
    if fq < NZ:
        return [(fq, fq + 1), (NZ + max(0, fq - NW), NZ + min(fq, NX))]
    return [(NZ, fq + 1)]


# ----------------------------------------------------------------------
# device program
# ----------------------------------------------------------------------
_CACHE = {}


def _build(n_blocks):
    nc = bacc.Bacc("TRN2", target_bir_lowering=False, debug=False,
                   num_devices=NCORE)
    x0_e = nc.declare_dram_parameter("x0", [TOK, D], F32, isOutput=False)
    xout_e = nc.declare_dram_parameter("xout", [TOK, D], F32, isOutput=True)
    ext = []
    for i in range(n_blocks):
        e = dict(
            wqkvo=nc.declare_dram_parameter(f"wqkvo{i}", [128, 16, D], BF16, isOutput=False),
            wg=nc.declare_dram_parameter(f"wg{i}", [128, 4, 2 * HID], BF16, isOutput=False),
            wf=nc.declare_dram_parameter(f"wf{i}", [128, 16, D], BF16, isOutput=False),
            bq=nc.declare_dram_parameter(f"bq{i}", [128, 4], F32, isOutput=False),
            bgl=nc.declare_dram_parameter(f"bgl{i}", [128, 32], F32, isOutput=False),
            bop=nc.declare_dram_parameter(f"bop{i}", [128, D], F32, isOutput=False),
            bff=nc.declare_dram_parameter(f"bff{i}", [128, D], F32, isOutput=False),
            tabs=nc.declare_dram_parameter(f"tabs{i}", [TOK, 6, D], F32, isOutput=False),
        )
        ext.append(e)

    with tile.TileContext(nc) as tc:
        with (
            tc.tile_pool(name="const", bufs=1) as cpool,
            tc.tile_pool(name="xp", bufs=2) as xpool,
            tc.tile_pool(name="wp", bufs=2) as wpool,
            tc.tile_pool(name="wg", bufs=1) as wgpool,
            tc.tile_pool(name="tb", bufs=1) as tbpool,
            tc.tile_pool(name="act", bufs=1) as apool,
            tc.tile_pool(name="sc", bufs=3) as scpool,
            tc.tile_pool(name="at", bufs=1) as atpool,
            tc.tile_pool(name="pt", bufs=4) as ptpool,
            tc.tile_pool(name="ot", bufs=3) as otpool,
            tc.tile_pool(name="ps", bufs=2, space="PSUM") as pspool,
            tc.tile_pool(name="ps2", bufs=2, space="PSUM") as ps2pool,
            tc.tile_pool(name="ps3", bufs=2, space="PSUM") as ps3pool,
            tc.tile_pool(name="dram", bufs=1, space="DRAM") as dpool,
        ):
            ident = cpool.tile([128, 128], BF16)
            make_identity(nc, ident[:])

            x = xpool.tile([128, 3, D], F32, tag="x")
            nc.sync.dma_start(x[:, 0:2, :], x0_e[0:256, :].rearrange("(t r) d -> r t d", r=128))
            nc.sync.dma_start(x[:107, 2, :], x0_e[256:363, :])

            for i in range(n_blocks):
                e = ext[i]
                # ---- load weights / tables ----
                wqkvo = wpool.tile([128, 16, D], BF16, tag="wqkvo")
                nc.sync.dma_start(wqkvo[:], e["wqkvo"][:])
                wg_sb = wgpool.tile([128, 4, 2 * HID], BF16, tag="wg")
                nc.sync.dma_start(wg_sb[:], e["wg"][:])
                wf_sb = wgpool.tile([128, 16, D], BF16, tag="wf")
                nc.sync.dma_start(wf_sb[:], e["wf"][:])
                bq_sb = cpool.tile([128, 4], F32, tag="bq")
                nc.sync.dma_start(bq_sb[:], e["bq"][:])
                bgl_sb = cpool.tile([128, 32], F32, tag="bgl")
                nc.sync.dma_start(bgl_sb[:], e["bgl"][:])
                bop_sb = cpool.tile([128, D], F32, tag="bop")
                nc.sync.dma_start(bop_sb[:], e["bop"][:])
                bff_sb = cpool.tile([128, D], F32, tag="bff")
                nc.sync.dma_start(bff_sb[:], e["bff"][:])
                tabs = tbpool.tile([128, 3, 6, D], F32, tag="tabs")
                nc.sync.dma_start(tabs[:, 0:2, :, :],
                                  e["tabs"][0:256].rearrange("(t r) s d -> r t s d", r=128))
                nc.sync.dma_start(tabs[:107, 2, :, :], e["tabs"][256:363])

                def ln_mod(src_x, si, ti_, xn_f32, xn_bf):
                    """xn = LN(src_x)*(tabs[si]) + tabs[ti_]; writes f32 + bf16."""
                    for tt, (r0, p_) in enumerate(TOKT):
                        xt = src_x[:p_, tt, :]
                        red = scpool.tile([128, 1], F32, tag="red")
                        nc.vector.reduce_sum(red[:p_], xt, axis=AX)
                        mu = scpool.tile([128, 1], F32, tag="mu")
                        nc.vector.tensor_scalar_mul(mu[:p_], red[:p_], 1.0 / D)
                        xc = scpool.tile([128, D], F32, tag="xc")
                        nc.vector.tensor_scalar(xc[:p_], xt, mu[:p_], None, op0=ALU.subtract)
                        sq = scpool.tile([128, D], F32, tag="sq")
                        ssq = scpool.tile([128, 1], F32, tag="ssq")
                        nc.scalar.activation(sq[:p_], xc[:p_], ACT.Square,
                                             accum_out=ssq[:p_])
                        std = scpool.tile([128, 1], F32, tag="std")
                        nc.scalar.activation(std[:p_], ssq[:p_], ACT.Sqrt,
                                             bias=1e-5, scale=1.0 / D)
                        rin = scpool.tile([128, 1], F32, tag="rin")
                        nc.vector.reciprocal(rin[:p_], std[:p_])
                        tmp = scpool.tile([128, D], F32, tag="lntmp")
                        nc.vector.scalar_tensor_tensor(
                            tmp[:p_], xc[:p_], rin[:p_], tabs[:p_, tt, si, :],
                            op0=ALU.mult, op1=ALU.mult)
                        nc.vector.tensor_add(xn_f32[:p_, tt, :], tmp[:p_],
                                             tabs[:p_, tt, ti_, :])
                        nc.vector.tensor_copy(xn_bf[:p_, tt, :], xn_f32[:p_, tt, :])

                def transpose_tok(xn_bf, xnT):
                    """xn_bf [128,3,D] bf16 -> xnT [128,4,TOK] bf16 (d-major)."""
                    for tt, (r0, p_) in enumerate(TOKT):
                        for kd in range(4):
                            pst = ps2pool.tile([128, 128], F32, tag="psS")
                            nc.tensor.transpose(pst[:, :p_],
                                                xn_bf[:p_, tt, kd * 128:(kd + 1) * 128],
                                                ident[:p_, :p_])
                            nc.vector.tensor_copy(xnT[:, kd, r0:r0 + p_], pst[:, :p_])

                # ---- mod1 ----
                xn1 = apool.tile([128, 3, D], F32, tag="xn1")
                xn1b = apool.tile([128, 3, D], BF16, tag="xn1b")
                ln_mod(x, 0, 1, xn1, xn1b)
                xnT = apool.tile([128, 4, TOK], BF16, tag="xnT")
                transpose_tok(xn1b, xnT)

                # ---- QKV ----
                qt_sb = apool.tile([128, 4, TOK], BF16, tag="qt")
                kt_sb = apool.tile([128, 4, TOK], BF16, tag="kt")
                for m in range(4):
                    psq = pspool.tile([128, TOK], F32, tag="psbig")
                    for k in range(4):
                        nc.tensor.matmul(psq[:], wqkvo[:, k, m * 128:(m + 1) * 128],
                                         xnT[:, k, :], start=(k == 0), stop=(k == 3))
                    nc.vector.tensor_scalar(qt_sb[:, m, :], psq[:], bq_sb[:, m:m + 1],
                                            None, op0=ALU.add)
                    psk = pspool.tile([128, TOK], F32, tag="psbig")
                    for k in range(4):
                        nc.tensor.matmul(psk[:], wqkvo[:, 4 + k, m * 128:(m + 1) * 128],
                                         xnT[:, k, :], start=(k == 0), stop=(k == 3))
                    nc.vector.tensor_copy(kt_sb[:, m, :], psk[:])
                v_sb = apool.tile([128, 3, D], BF16, tag="vsb")
                for tt, (r0, p_) in enumerate(TOKT):
                    psv = pspool.tile([128, D], F32, tag="psbig")
                    for k in range(4):
                        nc.tensor.matmul(psv[:p_], xnT[:, k, r0:r0 + p_],
                                         wqkvo[:, 8 + k, :], start=(k == 0), stop=(k == 3))
                    nc.vector.tensor_copy(v_sb[:p_, tt, :], psv[:p_])

                # ---- A2A fwd: send [8, 192, 363] (rows: 64 Q^T | 64 K^T | 64 V^T-layout) ----
                a2a_s = dpool.tile([NCORE, 192, TOK], BF16, tag=f"a2as{i}")
                a2a_r = dpool.tile([NCORE, 192, TOK], BF16, tag=f"a2ar{i}")
                for d in range(NCORE):
                    p0 = (64 * d) % 128
                    nc.sync.dma_start(a2a_s[d, 0:64, :], qt_sb[p0:p0 + 64, d // 2, :])
                    nc.sync.dma_start(a2a_s[d, 64:128, :], kt_sb[p0:p0 + 64, d // 2, :])
                    for tt, (r0, p_) in enumerate(TOKT):
                        nc.sync.dma_start(
                            a2a_s[d, 128:192, r0:r0 + p_].rearrange("j t -> t j"),
                            v_sb[:p_, tt, 64 * d:64 * (d + 1)])
                nc.gpsimd.collective_compute(
                    "AllToAll", ALU.bypass, replica_groups=[CORE_IDS],
                    ins=[a2a_s.opt()], outs=[a2a_r.opt()])

                # ---- assemble attention operands (for this core's head) ----
                qt_a = atpool.tile([64, NF, TPF], BF16, tag="qta")
                kt_a = atpool.tile([64, NF, S], BF16, tag="kta")
                ktl = atpool.tile([64, NF], BF16, tag="ktl")
                v_a = atpool.tile([128, 2 * NF, DH + 1], BF16, tag="va")
                v_l = atpool.tile([NF, DH + 1], BF16, tag="vl")
                nc.vector.memset(v_a[:, :, DH:DH + 1], 1.0)
                nc.vector.memset(v_l[:NF, DH:DH + 1], 1.0)
                for f in range(NF):
                    nc.sync.dma_start(
                        qt_a[:, f, 0:S],
                        a2a_r[:, 0:64, 33 * f:33 * f + 32].rearrange("s r j -> r s j"))
                    nc.sync.dma_start(
                        kt_a[:, f, :],
                        a2a_r[:, 64:128, 33 * f:33 * f + 32].rearrange("s r j -> r s j"))
                    for s_ in range(NCORE):
                        p0 = (32 * s_) % 128
                        nc.sync.dma_start(
                            v_a[p0:p0 + 32, 2 * f + (32 * s_) // 128, 0:DH],
                            a2a_r[s_, 128:192, 33 * f:33 * f + 32].rearrange("j t -> t j"))
                nc.sync.dma_start(qt_a[:, :, S], a2a_r[7, 0:64, 32::33])
                nc.sync.dma_start(ktl[:, :], a2a_r[7, 64:128, 32::33])
                nc.sync.dma_start(v_l[0:NF, 0:DH],
                                  a2a_r[7, 128:192, 32::33].rearrange("j f -> f j"))

                # ---- attention (this head, all 11 q-frames) ----
                bk_s = dpool.tile([NCORE, 65, TOK], BF16, tag=f"bks{i}")
                bk_r = dpool.tile([NCORE, 65, TOK], BF16, tag=f"bkr{i}")
                for fq in range(NF):
                    kvf = _kv_frames(fq)
                    n_av = 2 * len(kvf) + 1
                    ps_o = ps3pool.tile([65, TPF], F32, tag="psO")
                    av_i = 0
                    for fi in kvf:
                        for t2 in range(2):
                            ps_s = ps2pool.tile([128, TPF], F32, tag="psS")
                            nc.tensor.matmul(ps_s[:], kt_a[:, fi, 128 * t2:128 * (t2 + 1)],
                                             qt_a[:, fq, :], start=True, stop=True)
                            pt = ptpool.tile([128, TPF], BF16, tag="pt")
                            nc.scalar.activation(pt[:], ps_s[:], ACT.Exp, scale=0.125)
                            nc.tensor.matmul(ps_o[:], v_a[:, 2 * fi + t2, :], pt[:],
                                             start=(av_i == 0), stop=(av_i == n_av - 1))
                            av_i += 1
                    ps_l = ps2pool.tile([NF, TPF], F32, tag="psS")
                    nc.tensor.matmul(ps_l[:], ktl[:, :], qt_a[:, fq, :],
                                     start=True, stop=True)
                    pl = ptpool.tile([NF, TPF], BF16, tag="pt")
                    nc.vector.memset(pl[:NF], 0.0)
                    for (a_, b_) in _runs(fq):
                        if b_ > a_:
                            nc.scalar.activation(pl[a_:b_, :], ps_l[a_:b_, :],
                                                 ACT.Exp, scale=0.125)
                    nc.tensor.matmul(ps_o[:], v_l[:NF, :], pl[:NF],
                                     start=False, stop=True)
                    ot = otpool.tile([65, TPF], BF16, tag="ot")
                    nc.vector.tensor_copy(ot[:], ps_o[:])
                    rep = otpool.tile([65, 8], BF16, tag="rep")
                    nc.vector.tensor_copy(rep[:65, :], ot[:, S:S + 1].broadcast_to([65, 8]))
                    for d in range(NCORE):
                        nc.sync.dma_start(bk_s[d, :, 33 * fq:33 * fq + 32],
                                          ot[:, 32 * d:32 * d + 32])
                    nc.sync.dma_start(bk_s[:, :, 33 * fq + 32].rearrange("d r -> r d"),
                                      rep[:65, :])
                nc.gpsimd.collective_compute(
                    "AllToAll", ALU.bypass, replica_groups=[CORE_IDS],
                    ins=[bk_s.opt()], outs=[bk_r.opt()])

                # ---- gather attention output back, normalize by denominators ----
                xaT = apool.tile([128, 4, TOK], BF16, tag="xaT")
                xraw = apool.tile([128, 4, TOK], BF16, tag="xraw")
                for h in range(NH):
                    p0 = (64 * h) % 128
                    nc.sync.dma_start(xraw[p0:p0 + 64, h // 2, :], bk_r[h, 0:64, :])
                den = apool.tile([NH, TOK], BF16, tag="den")
                nc.sync.dma_start(den[:NH], bk_r[:, 64, :])
                rec = apool.tile([NH, TOK], F32, tag="rec")
                nc.vector.reciprocal(rec[:NH], den[:NH])
                for h in range(NH):
                    p0 = (64 * h) % 128
                    bc = scpool.tile([64, TOK], F32, tag="bcrec")
                    nc.gpsimd.partition_broadcast(bc[:64], rec[h:h + 1, :])
                    nc.vector.tensor_mul(xaT[p0:p0 + 64, h // 2, :],
                                         xraw[p0:p0 + 64, h // 2, :], bc[:64])

                # ---- Wo + gate1*xn1 + bo' ----
                x2 = apool.tile([128, 3, D], F32, tag="x2")
                for tt, (r0, p_) in enumerate(TOKT):
                    pso = pspool.tile([128, D], F32, tag="psbig")
                    for k in range(4):
                        nc.tensor.matmul(pso[:p_], xaT[:, k, r0:r0 + p_],
                                         wqkvo[:, 12 + k, :], start=(k == 0), stop=(k == 3))
                    g1x = scpool.tile([128, D], F32, tag="g1x")
                    nc.vector.tensor_mul(g1x[:p_], xn1[:p_, tt, :], tabs[:p_, tt, 2, :])
                    tmp = scpool.tile([128, D], F32, tag="wotmp")
                    nc.vector.tensor_add(tmp[:p_], pso[:p_], bop_sb[:p_])
                    nc.vector.tensor_add(x2[:p_, tt, :], tmp[:p_], g1x[:p_])

                # ---- mod2 ----
                xn2 = apool.tile([128, 3, D], F32, tag="xn1")       # reuse tag
                xn2b = apool.tile([128, 3, D], BF16, tag="xn1b")
                ln_mod(x2, 3, 4, xn2, xn2b)
                xn2T = apool.tile([128, 4, TOK], BF16, tag="xnT")
                transpose_tok(xn2b, xn2T)

                # ---- GEGLU ----
                h_sb = apool.tile([128, 16, TOK], BF16, tag="hsb")
                for mm in range(16):
                    psa = pspool.tile([128, TOK], F32, tag="psbig")
                    for k in range(4):
                        nc.tensor.matmul(psa[:], wg_sb[:, k, 128 * mm:128 * (mm + 1)],
                                         xn2T[:, k, :], start=(k == 0), stop=(k == 3))
                    psg = pspool.tile([128, TOK], F32, tag="psbig")
                    for k in range(4):
                        nc.tensor.matmul(psg[:], wg_sb[:, k, HID + 128 * mm:HID + 128 * (mm + 1)],
                                         xn2T[:, k, :], start=(k == 0), stop=(k == 3))
                    gel = scpool.tile([128, TOK], BF16, tag="gel")
                    nc.scalar.activation(gel[:], psg[:], ACT.Gelu,
                                         bias=bgl_sb[:, 16 + mm:17 + mm])
                    nc.vector.scalar_tensor_tensor(h_sb[:, mm, :], psa[:],
                                                   bgl_sb[:, mm:mm + 1], gel[:],
                                                   op0=ALU.add, op1=ALU.mult)

                # ---- FF out + gate2 ----
                x_new = xpool.tile([128, 3, D], F32, tag="x")
                for tt, (r0, p_) in enumerate(TOKT):
                    psf = pspool.tile([128, D], F32, tag="psbig")
                    for hh in range(16):
                        nc.tensor.matmul(psf[:p_], h_sb[:, hh, r0:r0 + p_],
                                         wf_sb[:, hh, :], start=(hh == 0), stop=(hh == 15))
                    tmp = scpool.tile([128, D], F32, tag="fftmp")
                    nc.vector.tensor_add(tmp[:p_], psf[:p_], bff_sb[:p_])
                    nc.vector.tensor_mul(x_new[:p_, tt, :], tmp[:p_], tabs[:p_, tt, 5, :])
                x = x_new

            nc.sync.dma_start(xout_e[0:256, :].rearrange("(t r) d -> r t d", r=128),
                              x[:, 0:2, :])
            nc.sync.dma_start(xout_e[256:363, :], x[:107, 2, :])
    nc.compile()
    return nc


# ----------------------------------------------------------------------
# host side
# ----------------------------------------------------------------------
def _silu(x):
    return x / (1.0 + np.exp(-x))


def _host_prep(inputs, n_blocks):
    f32 = np.float32
    z = np.asarray(inputs['z'], f32)
    frames = np.asarray(inputs['frames'], f32)
    actions = np.asarray(inputs['actions'])
    ts = np.asarray(inputs['ts'])

    def patch(xx):
        b, dur, c, h, w = xx.shape
        xx = xx.reshape(b, dur, c, h // P2, P2, w // P2, P2)
        xx = xx.transpose(0, 1, 3, 5, 2, 4, 6).reshape(b, dur, (h // P2) * (w // P2), c * P2 * P2)
        return xx @ np.asarray(inputs['W_patch'], f32) + np.asarray(inputs['b_patch'], f32)

    pe = np.asarray(inputs['pe_grid'], f32)
    zt = patch(z)[0] + pe[None]
    xt = patch(frames)[0] + pe[None]
    reg = np.asarray(inputs['registers'], f32)
    aemb = np.asarray(inputs['action_emb'], f32)
    temb = np.asarray(inputs['time_emb'], f32)
    a = aemb[actions[0]]

    ft = np.zeros((NF, TPF, D), f32)
    for f in range(NZ):
        ft[f, :S] = zt[f]
        ft[f, S] = reg[0]
    for f in range(NX):
        ft[NZ + f, :S] = xt[f]
        ft[NZ + f, S] = a[f]

    cond = np.zeros((NF, D), f32)
    for f in range(NZ):
        cond[f] = temb[ts[0, f]]
    for f in range(NX):
        cond[NZ + f] = temb[0]
    sc = _silu(cond)

    rep_idx = np.arange(TOK) // (OWN + 1)   # frame of each per-core row
    blocks = []
    for i in range(n_blocks):
        m1 = sc @ np.asarray(inputs['W_mod1'][i], f32) + np.asarray(inputs['b_mod1'][i], f32)
        s1, t1 = np.split(m1, 2, -1)
        m2 = sc @ np.asarray(inputs['W_mod2'][i], f32) + np.asarray(inputs['b_mod2'][i], f32)
        s2, t2 = np.split(m2, 2, -1)
        g1 = cond @ np.asarray(inputs['W_g1'][i], f32) + np.asarray(inputs['b_g1'][i], f32)
        g2 = cond @ np.asarray(inputs['W_g2'][i], f32) + np.asarray(inputs['b_g2'][i], f32)
        bo_p = (np.asarray(inputs['b_o'][i], f32)
                + np.asarray(inputs['b_v'][i], f32) @ np.asarray(inputs['W_o'][i], f32))
        tabs = np.stack([1.0 + s1, t1, g1, 1.0 + s2, t2, g2], 1)  # [NF, 6, D]
        tabs_tok = tabs[rep_idx]                                   # [TOK, 6, D]

        def chunk(w, kparts):
            # [K, N] -> [128, K//128, N]
            K, N = w.shape
            return np.ascontiguousarray(
                np.asarray(w, f32).reshape(kparts, 128, N).swapaxes(0, 1)).astype(BF)

        wq = chunk(np.asarray(inputs['W_q'][i]), 4)
        wk = chunk(np.asarray(inputs['W_k'][i]), 4)
        wv = chunk(np.asarray(inputs['W_v'][i]), 4)
        wo = chunk(np.asarray(inputs['W_o'][i]), 4)
        wqkvo = np.concatenate([wq, wk, wv, wo], 1)               # [128, 16, 512]
        blocks.append(dict(
            wqkvo=wqkvo,
            wg=chunk(np.asarray(inputs['W_geglu'][i]), 4),
            wf=chunk(np.asarray(inputs['W_ffout'][i]), 16),
            bq=np.ascontiguousarray(np.asarray(inputs['b_q'][i], f32).reshape(4, 128).T),
            bgl=np.ascontiguousarray(np.asarray(inputs['b_geglu'][i], f32).reshape(32, 128).T),
            bop=np.tile(bo_p[None], (128, 1)).astype(f32),
            bff=np.tile(np.asarray(inputs['b_ffout'][i], f32)[None], (128, 1)),
            tabs=np.ascontiguousarray(tabs_tok, f32),
        ))
    return ft, blocks


def kernel(**inputs):
    import os
    n_blocks = int(os.environ.get("KERNEL_NBLOCKS", NB))
    ft, blocks = _host_prep(inputs, n_blocks)

    # shard x
    in_maps = []
    for c in range(NCORE):
        xo = np.zeros((NF, OWN + 1, D), np.float32)
        for f in range(NF):
            xo[f, :OWN] = ft[f, OWN * c:OWN * (c + 1)]
            xo[f, OWN] = ft[f, S]
        m = {"x0": xo.reshape(TOK, D)}
        for i in range(n_blocks):
            for k, v in blocks[i].items():
                m[f"{k}{i}"] = v
        in_maps.append(m)

    if n_blocks not in _CACHE:
        _CACHE[n_blocks] = _build(n_blocks)
    nc = _CACHE[n_blocks]
    res = run_bass_kernel_spmd(nc, in_maps, CORE_IDS)

    # unshard
    out = np.zeros((NF, TPF, D), np.float32)
    for c in range(NCORE):
        xo = res.results[c]["xout"].reshape(NF, OWN + 1, D)
        for f in range(NF):
            out[f, OWN * c:OWN * (c + 1)] = xo[f, :OWN]
    x0 = res.results[0]["xout"].reshape(NF, OWN + 1, D)
    for f in range(NF):
        out[f, S] = x0[f, OWN]

    # unpatch (host)
    f32 = np.float32
    zr = out[:NZ, :S]
    y = zr @ np.asarray(inputs['W_unpatch'], f32) + np.asarray(inputs['b_unpatch'], f32)
    y = y.reshape(1, NZ, HH // P2, WW // P2, C, P2, P2)
    y = y.transpose(0, 1, 4, 2, 5, 3, 6).reshape(1, NZ, C, HH, WW)
    return np.ascontiguousarray(y.astype(np.float32))


# revision 13
# speedup vs baseline: 1.0092x; 1.0092x over previous
"""Trainium2 Bass kernel for nn_CausalDit (sparse frame-causal DiT).

Sharding over 8 NeuronCores (SPMD, one program):
- Token space: 11 frames (6 noisy "zr" + 5 clean "xa"), 257 tokens each
  (256 patch tokens + 1 register/action token).
- Dense compute (LN/AdaLN-mod, QKV, Wo, GEGLU FFN, gates) is sharded by
  tokens: core c owns rows [32c, 32c+32) of every frame; the per-frame
  257th token ("leftover") is computed replicated on every core.
  Per-core stream: [11 frames x 33 rows, 512] = [363, 512].
- Attention is sharded by head (8 heads <-> 8 cores), block-sparse at
  frame granularity (no masks): an AllToAll redistributes Q^T/K^T/V from
  token-shards to head-shards, each core runs all 35 frame-pair attention
  units for its head (softmax without max-subtraction; denominator via a
  ones-column appended to V, normalization on the attention side), and a
  second AllToAll returns normalized per-head outputs to token owners.
- Matmuls run as float32r (full PE rate for free-dim >= 256, near-fp32
  precision) with fp32 PSUM accumulation; storage is plain fp32 with
  zero-copy bitcasts at matmul call sites.
- Host (numpy, fp32) does only tiny prep: patchify + patch matmul,
  embedding gathers, per-frame AdaLN scale/shift/gate tables, bias folds
  (b_k dropped - softmax-shift-invariant; b_v folded into b_o), and the
  final unpatch matmul.
"""
import numpy as np

import concourse.bass as bass
import concourse.mybir as mybir
import concourse.tile as tile
from concourse import bacc
from concourse.bass_utils import run_bass_kernel_spmd
from concourse.masks import make_identity

# ---- model constants (hardcoded from the problem spec) ----
P2 = 2; NH = 8; NW = 4; NB = 6; D = 512; HID = 2048
HH = 32; WW = 32; C = 3; DUR = 6
DH = D // NH          # 64
NZ = DUR              # 6 zr frames
NX = DUR - 1          # 5 xa frames
NF = NZ + NX          # 11 frames
S = (HH // P2) * (WW // P2)   # 256
TPF = S + 1           # 257 tokens/frame
NCORE = 8
OWN = 32              # owned rows per frame per core
TOK = NF * (OWN + 1)  # 363 real rows per core
TOKP = TOK + 1        # padded to even (364) for fp32r matmul rules
TPQ = TPF + 1         # q columns padded to 258
NFP = NF + 1          # leftover rows padded to 12
VA = DH + 2           # V_aug cols: 64 v + 1 ones + 1 pad = 66
CORE_IDS = list(range(NCORE))
TOKT = [(0, 128), (128, 128), (256, 108)]   # token tiles of 364
F32 = mybir.dt.float32
F32R = mybir.dt.float32r
AX = mybir.AxisListType.X
ALU = mybir.AluOpType
ACTF = mybir.ActivationFunctionType


def _kv_frames(fq):
    """Global kv-frame indices for q-frame fq (zr: 0..5, xa: 6..10)."""
    if fq < NZ:
        return [fq] + [NZ + j for j in range(max(0, fq - NW), min(fq, NX))]
    return list(range(NZ, fq + 1))


def _R(ap):
    return ap.bitcast(F32R)


_CACHE = {}
LAST_RESULT = None


def _build(n_blocks):
    nc = bacc.Bacc("TRN2", target_bir_lowering=False, debug=False,
                   num_devices=NCORE)
    x0_e = nc.declare_dram_parameter("x0", [TOKP, D], F32, isOutput=False)
    lb_e = nc.declare_dram_parameter("lbias", [NFP, NFP], F32, isOutput=False)
    xout_e = nc.declare_dram_parameter("xout", [TOKP, D], F32, isOutput=True)
    ext = []
    for i in range(n_blocks):
        e = dict(
            wqkvo=nc.declare_dram_parameter(f"wqkvo{i}", [128, 16, D], F32R, isOutput=False),
            wg=nc.declare_dram_parameter(f"wg{i}", [128, 4, 2 * HID], F32R, isOutput=False),
            wf=nc.declare_dram_parameter(f"wf{i}", [128, 16, D], F32R, isOutput=False),
            bq=nc.declare_dram_parameter(f"bq{i}", [128, 4], F32, isOutput=False),
            bgl=nc.declare_dram_parameter(f"bgl{i}", [128, 32], F32, isOutput=False),
            bop=nc.declare_dram_parameter(f"bop{i}", [128, D], F32, isOutput=False),
            bff=nc.declare_dram_parameter(f"bff{i}", [128, D], F32, isOutput=False),
            tabs=nc.declare_dram_parameter(f"tabs{i}", [TOKP, 6, D], F32, isOutput=False),
        )
        ext.append(e)

    with tile.TileContext(nc) as tc:
        with (
            tc.tile_pool(name="const", bufs=1) as cpool,
            tc.tile_pool(name="xp", bufs=2) as xpool,
            tc.tile_pool(name="wp", bufs=1) as wpool,
            tc.tile_pool(name="wgs", bufs=2) as wgpool,
            tc.tile_pool(name="tb", bufs=2) as tbpool,
            tc.tile_pool(name="act", bufs=1) as apool,
            tc.tile_pool(name="hp", bufs=2) as hpool,
            tc.tile_pool(name="sc", bufs=2) as scpool,
            tc.tile_pool(name="at", bufs=1) as atpool,
            tc.tile_pool(name="pt", bufs=3) as ptpool,
            tc.tile_pool(name="ot", bufs=3) as otpool,
            tc.tile_pool(name="ps", bufs=2, space="PSUM") as pspool,
            tc.tile_pool(name="ps2", bufs=2, space="PSUM") as ps2pool,
            tc.tile_pool(name="psOf", bufs=3, space="PSUM") as psofpool,
            tc.tile_pool(name="dram", bufs=1, space="DRAM") as dpool,
        ):
            ident32 = cpool.tile([128, 128], F32)
            make_identity(nc, ident32[:])
            ident = cpool.tile([128, 128], F32R)
            nc.vector.tensor_copy(ident[:], ident32[:])
            eps = cpool.tile([128, 1], F32)
            nc.vector.memset(eps[:], 1e-5)
            lbias = cpool.tile([NFP, NFP], F32)
            nc.sync.dma_start(lbias[:NFP], lb_e[:])
            aug = cpool.tile([128, VA], F32)       # [0]*64 | 1.0 | 0.0
            nc.vector.memset(aug[:], 0.0)
            nc.vector.memset(aug[:, DH:DH + 1], 1.0)

            x = xpool.tile([128, 3, D], F32, tag="x")
            nc.sync.dma_start(x[:, 0:2, :], x0_e[0:256, :].rearrange("(t r) d -> r t d", r=128))
            nc.sync.dma_start(x[:108, 2, :], x0_e[256:364, :])

            for i in range(n_blocks):
                e = ext[i]
                wqkvo = wpool.tile([128, 16, D], F32R, tag="wqkvo")
                nc.sync.dma_start(wqkvo[:], e["wqkvo"][:])
                bq_sb = cpool.tile([128, 4], F32, tag="bq")
                nc.sync.dma_start(bq_sb[:], e["bq"][:])
                bgl_sb = cpool.tile([128, 32], F32, tag="bgl")
                nc.sync.dma_start(bgl_sb[:], e["bgl"][:])
                bop_sb = cpool.tile([128, D], F32, tag="bop")
                nc.sync.dma_start(bop_sb[:], e["bop"][:])
                bff_sb = cpool.tile([128, D], F32, tag="bff")
                nc.sync.dma_start(bff_sb[:], e["bff"][:])

                def load_tab(v, e=e):
                    t = tbpool.tile([128, 3, D], F32, tag="tabv")
                    nc.sync.dma_start(
                        t[:, 0:2, :],
                        e["tabs"][0:256, v, :].rearrange("(t r) d -> r t d", r=128))
                    nc.sync.dma_start(t[:108, 2, :], e["tabs"][256:364, v, :])
                    return t

                def ln_mod(src_x, s_t, t_t, xn_f32):
                    """xn = LN(src_x)*s_t + t_t (fp32)."""
                    for tt, (r0, p_) in enumerate(TOKT):
                        xt = src_x[:p_, tt, :]
                        red = scpool.tile([128, 1], F32, tag="red")
                        nc.vector.reduce_sum(red[:p_], xt, axis=AX)
                        mu = scpool.tile([128, 1], F32, tag="mu")
                        nc.vector.tensor_scalar_mul(mu[:p_], red[:p_], 1.0 / D)
                        xc = scpool.tile([128, D], F32, tag="xc")
                        nc.vector.tensor_scalar(xc[:p_], xt, mu[:p_], None, op0=ALU.subtract)
                        sq = scpool.tile([128, D], F32, tag="lntmp")
                        ssq = scpool.tile([128, 1], F32, tag="ssq")
                        nc.scalar.activation(sq[:p_], xc[:p_], ACTF.Square,
                                             accum_out=ssq[:p_])
                        std = scpool.tile([128, 1], F32, tag="std")
                        nc.scalar.activation(std[:p_], ssq[:p_], ACTF.Sqrt,
                                             bias=eps[:p_], scale=1.0 / D)
                        rin = scpool.tile([128, 1], F32, tag="rin")
                        nc.vector.reciprocal(rin[:p_], std[:p_])
                        tmp = scpool.tile([128, D], F32, tag="lntmp")
                        nc.vector.scalar_tensor_tensor(
                            tmp[:p_], xc[:p_], rin[:p_], s_t[:p_, tt, :],
                            op0=ALU.mult, op1=ALU.mult)
                        nc.vector.tensor_add(xn_f32[:p_, tt, :], tmp[:p_],
                                             t_t[:p_, tt, :])

                def transpose_tok(xn_f32, xnT):
                    """xn [128,3,D] f32 -> xnT [128,4,TOK] f32 (d-major)."""
                    for tt, (r0, p_) in enumerate(TOKT):
                        for kd in range(4):
                            pst = ps2pool.tile([128, 128], F32R, tag="psS")
                            nc.tensor.transpose(pst[:, :p_],
                                                xn_f32[:p_, tt, kd * 128:(kd + 1) * 128],
                                                ident[:p_, :p_])
                            nc.vector.tensor_copy(xnT[:, kd, r0:r0 + p_],
                                                  pst[:, :p_].bitcast(F32))

                # ---- mod1 ----
                s1p_t = load_tab(0)
                t1_t = load_tab(1)
                xn1 = apool.tile([128, 3, D], F32R, tag="xn1")
                ln_mod(x, s1p_t, t1_t, xn1)
                xnT = apool.tile([128, 4, TOKP], F32R, tag="xnT")
                transpose_tok(xn1, xnT)

                # ---- QKV ----
                qt_sb = apool.tile([128, 4, TOKP], F32R, tag="qt")
                kt_sb = apool.tile([128, 4, TOKP], F32R, tag="big1")
                for m in range(4):
                    psq = pspool.tile([128, TOKP], F32, tag="psbig")
                    for k in range(4):
                        nc.tensor.matmul(psq[:], wqkvo[:, k, m * 128:(m + 1) * 128],
                                         xnT[:, k, :], start=(k == 0), stop=(k == 3))
                    nc.vector.tensor_scalar(qt_sb[:, m, :], psq[:], bq_sb[:, m:m + 1],
                                            None, op0=ALU.add)
                    psk = pspool.tile([128, TOKP], F32, tag="psbig")
                    for k in range(4):
                        nc.tensor.matmul(psk[:], wqkvo[:, 4 + k, m * 128:(m + 1) * 128],
                                         xnT[:, k, :], start=(k == 0), stop=(k == 3))
                    nc.vector.tensor_copy(kt_sb[:, m, :], psk[:])
                v_sb = apool.tile([128, 3, D], F32R, tag="big2")
                for tt, (r0, p_) in enumerate(TOKT):
                    psv = pspool.tile([128, D], F32, tag="psbig")
                    for k in range(4):
                        nc.tensor.matmul(psv[:p_], xnT[:, k, r0:r0 + p_],
                                         wqkvo[:, 8 + k, :], start=(k == 0), stop=(k == 3))
                    nc.vector.tensor_copy(v_sb[:p_, tt, :], psv[:p_])

                # ---- A2A fwd: [8, 192, 363] f32 (rows: 64 Q^T | 64 K^T | 64 V^T-layout) ----
                a2a_s = dpool.tile([NCORE, 192, TOKP], F32R, tag=f"a2as{i}")
                a2a_r = dpool.tile([NCORE, 192, TOKP], F32R, tag=f"a2ar{i}")
                for d in range(NCORE):
                    p0 = (64 * d) % 128
                    nc.sync.dma_start(a2a_s[d, 0:64, :], qt_sb[p0:p0 + 64, d // 2, :])
                    nc.sync.dma_start(a2a_s[d, 64:128, :], kt_sb[p0:p0 + 64, d // 2, :])
                    for tt, (r0, p_) in enumerate(TOKT):
                        nc.sync.dma_start(
                            a2a_s[d, 128:192, r0:r0 + p_].rearrange("j t -> t j"),
                            v_sb[:p_, tt, 64 * d:64 * (d + 1)])
                import os as _os
                if _os.environ.get("KERNEL_SKIP_COLL"):
                    nc.sync.dma_start(a2a_r[:], a2a_s[:])
                else:
                    nc.gpsimd.collective_compute(
                        "AllToAll", ALU.bypass, replica_groups=[CORE_IDS],
                        ins=[a2a_s.opt()], outs=[a2a_r.opt()])

                # ---- assemble attention operands (this core's head) ----
                qt_a = atpool.tile([64, NF, TPQ], F32R, tag="qta")
                kt_a = atpool.tile([64, NF, S], F32R, tag="kta")
                ktl = atpool.tile([64, NFP], F32R, tag="ktl")
                v_a = atpool.tile([128, 2 * NF, VA], F32R, tag="va")
                v_l = atpool.tile([NFP, VA], F32R, tag="vl")
                nc.vector.tensor_copy(v_a[:, :, DH:DH + 2],
                                      aug[:, None, DH:DH + 2].broadcast_to([128, 2 * NF, 2]))
                nc.vector.tensor_copy(v_l[:NFP, :], aug[:NFP, :])
                nc.vector.tensor_copy(ktl[:64, :], aug[:64, 0:NFP])
                nc.vector.tensor_copy(qt_a[:, :, TPF:TPQ],
                                      aug[:64, None, 0:1].broadcast_to([64, NF, 1]))
                for f in range(NF):
                    nc.sync.dma_start(
                        qt_a[:, f, 0:S],
                        a2a_r[:, 0:64, 33 * f:33 * f + 32].rearrange("s r j -> r s j"))
                    nc.sync.dma_start(
                        kt_a[:, f, :],
                        a2a_r[:, 64:128, 33 * f:33 * f + 32].rearrange("s r j -> r s j"))
                    for s_ in range(NCORE):
                        p0 = (32 * s_) % 128
                        nc.sync.dma_start(
                            v_a[p0:p0 + 32, 2 * f + (32 * s_) // 128, 0:DH],
                            a2a_r[s_, 128:192, 33 * f:33 * f + 32].rearrange("j t -> t j"))
                nc.sync.dma_start(qt_a[:, :, S], a2a_r[7, 0:64, 32::33])
                nc.sync.dma_start(ktl[:, 0:NF], a2a_r[7, 64:128, 32::33])
                nc.sync.dma_start(v_l[0:NF, 0:DH],
                                  a2a_r[7, 128:192, 32::33].rearrange("j f -> f j"))

                # ---- attention (this head, all 11 q-frames) ----
                bk_s = dpool.tile([NCORE, 64, TOKP], F32R, tag=f"bks{i}")
                bk_r = dpool.tile([NCORE, 64, TOKP], F32R, tag=f"bkr{i}")
                for fq in range(NF):
                    kvf = _kv_frames(fq)
                    n_av = 2 * len(kvf) + 1
                    ps_o = psofpool.tile([VA, TPQ], F32, tag="psOf")
                    av_i = 0
                    for fi in kvf:
                        for t2 in range(2):
                            ps_s = ps2pool.tile([128, TPQ], F32, tag="psS")
                            nc.tensor.matmul(ps_s[:], kt_a[:, fi, 128 * t2:128 * (t2 + 1)],
                                             qt_a[:, fq, :], start=True, stop=True)
                            pt = ptpool.tile([128, TPQ], F32R, tag="pt")
                            nc.scalar.activation(pt[:], ps_s[:], ACTF.Exp, scale=0.125)
                            nc.tensor.matmul(ps_o[:], v_a[:, 2 * fi + t2, :], pt[:],
                                             start=(av_i == 0), stop=(av_i == n_av - 1))
                            av_i += 1
                    ps_l = ps2pool.tile([NFP, TPQ], F32, tag="psS")
                    nc.tensor.matmul(ps_l[:], ktl[:, :], qt_a[:, fq, :],
                                     start=True, stop=True)
                    pl = ptpool.tile([NFP, TPQ], F32R, tag="pt")
                    nc.scalar.activation(pl[:NFP, :], ps_l[:NFP, :], ACTF.Exp,
                                         scale=0.125, bias=lbias[:NFP, fq:fq + 1])
                    nc.tensor.matmul(ps_o[:], v_l[:NFP, :], pl[:NFP],
                                     start=False, stop=True)
                    rc = scpool.tile([1, TPQ], F32, tag="rc")
                    nc.vector.reciprocal(rc[0:1], ps_o[64:65, :])
                    bc = scpool.tile([64, TPQ], F32, tag="bcrec")
                    nc.gpsimd.partition_broadcast(bc[:64], rc[0:1, :])
                    ot = otpool.tile([64, TPQ], F32R, tag="ot")
                    nc.vector.tensor_mul(ot[:64], ps_o[0:64, :], bc[:64])
                    rep = otpool.tile([64, 8], F32R, tag="rep")
                    nc.vector.tensor_copy(rep[:64, :], ot[:64, S:S + 1].broadcast_to([64, 8]).bitcast(F32))
                    for d in range(NCORE):
                        nc.sync.dma_start(bk_s[d, :, 33 * fq:33 * fq + 32],
                                          ot[:64, 32 * d:32 * d + 32])
                    nc.sync.dma_start(bk_s[:, :, 33 * fq + 32].rearrange("d r -> r d"),
                                      rep[:64, :])
                if _os.environ.get("KERNEL_SKIP_COLL"):
                    nc.sync.dma_start(bk_r[:], bk_s[:])
                else:
                    nc.gpsimd.collective_compute(
                        "AllToAll", ALU.bypass, replica_groups=[CORE_IDS],
                        ins=[bk_s.opt()], outs=[bk_r.opt()])

                # ---- gather attention output back (already normalized) ----
                xaT = apool.tile([128, 4, TOKP], F32R, tag="qt")
                for h in range(NH):
                    p0 = (64 * h) % 128
                    nc.sync.dma_start(xaT[p0:p0 + 64, h // 2, :], bk_r[h, :, :])

                # ---- Wo + gate1*xn1 + bo' ----
                g1_t = load_tab(2)
                x2 = apool.tile([128, 3, D], F32, tag="big1")
                for tt, (r0, p_) in enumerate(TOKT):
                    pso = pspool.tile([128, D], F32, tag="psbig")
                    for k in range(4):
                        nc.tensor.matmul(pso[:p_], xaT[:, k, r0:r0 + p_],
                                         wqkvo[:, 12 + k, :], start=(k == 0), stop=(k == 3))
                    g1x = scpool.tile([128, D], F32, tag="g1x")
                    nc.vector.tensor_mul(g1x[:p_], xn1[:p_, tt, :].bitcast(F32), g1_t[:p_, tt, :])
                    tmp = scpool.tile([128, D], F32, tag="wotmp")
                    nc.vector.tensor_add(tmp[:p_], pso[:p_], bop_sb[:p_])
                    nc.vector.tensor_add(x2[:p_, tt, :], tmp[:p_], g1x[:p_])

                # ---- mod2 ----
                s2p_t = load_tab(3)
                t2_t = load_tab(4)
                xn2 = apool.tile([128, 3, D], F32R, tag="xn1")
                ln_mod(x2, s2p_t, t2_t, xn2)
                xn2T = apool.tile([128, 4, TOKP], F32R, tag="xnT")
                transpose_tok(xn2, xn2T)

                # ---- GEGLU + FF out, streamed in 4 weight pieces ----
                g2_t = load_tab(5)
                psf = [psofpool.tile([128, D], F32, tag="psOf", name=f"psf{i}_{tt}")
                       for tt in range(3)]
                for p in range(4):
                    wgp = wgpool.tile([128, 4, 1024], F32R, tag="wgp")
                    nc.sync.dma_start(wgp[:], e["wg"][:, :, 1024 * p:1024 * (p + 1)])
                    wfp = wgpool.tile([128, 4, D], F32R, tag="wfp")
                    nc.sync.dma_start(wfp[:], e["wf"][:, 4 * p:4 * (p + 1), :])
                    for j in range(4):
                        mm = 4 * p + j
                        psa = pspool.tile([128, TOKP], F32, tag="psbig")
                        for k in range(4):
                            nc.tensor.matmul(psa[:], wgp[:, k, 256 * j:256 * j + 128],
                                             _R(xn2T[:, k, :]), start=(k == 0), stop=(k == 3))
                        psg = pspool.tile([128, TOKP], F32, tag="psbig")
                        for k in range(4):
                            nc.tensor.matmul(psg[:], wgp[:, k, 256 * j + 128:256 * j + 256],
                                             _R(xn2T[:, k, :]), start=(k == 0), stop=(k == 3))
                        gel = scpool.tile([128, TOKP], F32, tag="gel")
                        nc.scalar.activation(gel[:], psg[:], ACTF.Gelu,
                                             bias=bgl_sb[:, 2 * mm + 1:2 * mm + 2])
                        hp = hpool.tile([128, TOKP], F32R, tag="hp")
                        nc.vector.scalar_tensor_tensor(hp[:], psa[:],
                                                       bgl_sb[:, 2 * mm:2 * mm + 1], gel[:],
                                                       op0=ALU.add, op1=ALU.mult)
                        for tt, (r0, p_) in enumerate(TOKT):
                            nc.tensor.matmul(psf[tt][:p_], hp[:, r0:r0 + p_],
                                             wfp[:, j, :],
                                             start=(mm == 0), stop=(mm == 15))

                x_new = xpool.tile([128, 3, D], F32, tag="x")
                for tt, (r0, p_) in enumerate(TOKT):
                    tmp = scpool.tile([128, D], F32, tag="fftmp")
                    nc.vector.tensor_add(tmp[:p_], psf[tt][:p_], bff_sb[:p_])
                    nc.vector.tensor_mul(x_new[:p_, tt, :], tmp[:p_], g2_t[:p_, tt, :])
                x = x_new
                if not _os.environ.get("KERNEL_NO_BLOCK_BARRIER"):
                    # scheduling barrier between blocks: without it the
                    # cross-block overlap of collectives deadlocks on HW
                    tc.strict_bb_all_engine_barrier()

            nc.sync.dma_start(xout_e[0:256, :].rearrange("(t r) d -> r t d", r=128),
                              x[:, 0:2, :])
            nc.sync.dma_start(xout_e[256:364, :], x[:108, 2, :])
    nc.compile()
    return nc


# ----------------------------------------------------------------------
# host side
# ----------------------------------------------------------------------
def _silu(x):
    return x / (1.0 + np.exp(-x))


def _host_prep(inputs, n_blocks):
    f32 = np.float32
    z = np.asarray(inputs['z'], f32)
    frames = np.asarray(inputs['frames'], f32)
    actions = np.asarray(inputs['actions'])
    ts = np.asarray(inputs['ts'])

    def patch(xx):
        b, dur, c, h, w = xx.shape
        xx = xx.reshape(b, dur, c, h // P2, P2, w // P2, P2)
        xx = xx.transpose(0, 1, 3, 5, 2, 4, 6).reshape(b, dur, (h // P2) * (w // P2), c * P2 * P2)
        return xx @ np.asarray(inputs['W_patch'], f32) + np.asarray(inputs['b_patch'], f32)

    pe = np.asarray(inputs['pe_grid'], f32)
    zt = patch(z)[0] + pe[None]
    xt = patch(frames)[0] + pe[None]
    reg = np.asarray(inputs['registers'], f32)
    aemb = np.asarray(inputs['action_emb'], f32)
    temb = np.asarray(inputs['time_emb'], f32)
    a = aemb[actions[0]]

    ft = np.zeros((NF, TPF, D), f32)
    for f in range(NZ):
        ft[f, :S] = zt[f]
        ft[f, S] = reg[0]
    for f in range(NX):
        ft[NZ + f, :S] = xt[f]
        ft[NZ + f, S] = a[f]

    cond = np.zeros((NF, D), f32)
    for f in range(NZ):
        cond[f] = temb[ts[0, f]]
    for f in range(NX):
        cond[NZ + f] = temb[0]
    sc = _silu(cond)

    rep_idx = np.arange(TOK) // (OWN + 1)
    blocks = []
    for i in range(n_blocks):
        m1 = sc @ np.asarray(inputs['W_mod1'][i], f32) + np.asarray(inputs['b_mod1'][i], f32)
        s1, t1 = np.split(m1, 2, -1)
        m2 = sc @ np.asarray(inputs['W_mod2'][i], f32) + np.asarray(inputs['b_mod2'][i], f32)
        s2, t2 = np.split(m2, 2, -1)
        g1 = cond @ np.asarray(inputs['W_g1'][i], f32) + np.asarray(inputs['b_g1'][i], f32)
        g2 = cond @ np.asarray(inputs['W_g2'][i], f32) + np.asarray(inputs['b_g2'][i], f32)
        bo_p = (np.asarray(inputs['b_o'][i], f32)
                + np.asarray(inputs['b_v'][i], f32) @ np.asarray(inputs['W_o'][i], f32))
        tabs = np.stack([1.0 + s1, t1, g1, 1.0 + s2, t2, g2], 1)
        tabs_tok = tabs[rep_idx]

        def chunk(w, kparts):
            K, N = w.shape
            return np.ascontiguousarray(
                np.asarray(w, f32).reshape(kparts, 128, N).swapaxes(0, 1))

        wq = chunk(np.asarray(inputs['W_q'][i]), 4)
        wk = chunk(np.asarray(inputs['W_k'][i]), 4)
        wv = chunk(np.asarray(inputs['W_v'][i]), 4)
        wo = chunk(np.asarray(inputs['W_o'][i]), 4)
        wqkvo = np.concatenate([wq, wk, wv, wo], 1)

        # interleave a/g columns of W_geglu so each 256-col group is (a_mm|g_mm)
        wg = chunk(np.asarray(inputs['W_geglu'][i]), 4)        # [128, 4, 4096]
        wg4 = wg.reshape(128, 4, 2, 16, 128)                   # [., ., a/g, mm, col]
        wg_i = np.ascontiguousarray(
            wg4.transpose(0, 1, 3, 2, 4).reshape(128, 4, 4096))
        bg = np.asarray(inputs['b_geglu'][i], f32).reshape(2, 16, 128)
        bgl = np.ascontiguousarray(
            bg.transpose(2, 1, 0).reshape(128, 32))            # [128, 32] cols (2mm, 2mm+1)

        blocks.append(dict(
            wqkvo=wqkvo,
            wg=wg_i,
            wf=chunk(np.asarray(inputs['W_ffout'][i]), 16),
            bq=np.ascontiguousarray(np.asarray(inputs['b_q'][i], f32).reshape(4, 128).T),
            bgl=bgl,
            bop=np.tile(bo_p[None], (128, 1)).astype(f32),
            bff=np.tile(np.asarray(inputs['b_ffout'][i], f32)[None], (128, 1)),
            tabs=np.concatenate([tabs_tok, np.zeros((1, 6, D), f32)], 0),
        ))
    return ft, blocks


def kernel(**inputs):
    import os
    n_blocks = int(os.environ.get("KERNEL_NBLOCKS", NB))
    ft, blocks = _host_prep(inputs, n_blocks)

    lb = np.full((NFP, NFP), -30.0, np.float32)
    for fq in range(NF):
        for kf in _kv_frames(fq):
            lb[kf, fq] = 0.0

    in_maps = []
    for c in range(NCORE):
        xo = np.zeros((NF, OWN + 1, D), np.float32)
        for f in range(NF):
            xo[f, :OWN] = ft[f, OWN * c:OWN * (c + 1)]
            xo[f, OWN] = ft[f, S]
        x0p = np.zeros((TOKP, D), np.float32)
        x0p[:TOK] = xo.reshape(TOK, D)
        m = {"x0": x0p, "lbias": lb}
        for i in range(n_blocks):
            for k, v in blocks[i].items():
                m[f"{k}{i}"] = v
        in_maps.append(m)

    if n_blocks not in _CACHE:
        _CACHE[n_blocks] = _build(n_blocks)
    nc = _CACHE[n_blocks]
    trace = bool(os.environ.get("KERNEL_TRACE"))
    res = run_bass_kernel_spmd(nc, in_maps, CORE_IDS, trace=trace)
    global LAST_RESULT
    LAST_RESULT = res

    out = np.zeros((NF, TPF, D), np.float32)
    for c in range(NCORE):
        xo = res.results[c]["xout"][:TOK].reshape(NF, OWN + 1, D)
        for f in range(NF):
            out[f, OWN * c:OWN * (c + 1)] = xo[f, :OWN]
    x0 = res.results[0]["xout"][:TOK].reshape(NF, OWN + 1, D)
    for f in range(NF):
        out[f, S] = x0[f, OWN]

    f32 = np.float32
    zr = out[:NZ, :S]
    y = zr @ np.asarray(inputs['W_unpatch'], f32) + np.asarray(inputs['b_unpatch'], f32)
    y = y.reshape(1, NZ, HH // P2, WW // P2, C, P2, P2)
    y = y.transpose(0, 1, 4, 2, 5, 3, 6).reshape(1, NZ, C, HH, WW)
    return np.ascontiguousarray(y.astype(np.float32))
